# revision 1
# baseline (speedup 1.0000x reference)
"""CPC loss (nn_CPCLossV2) Trainium2 Bass kernel.

Problem: n=4096 groups x k=4 rows of h=256 embeddings.
  hist_x[g]  = rows 4g..4g+2 concat -> [n, 768]
  hist_y[g]  = row 4g+3             -> [n, 256]
  predicts   = hist_x @ W + b       -> [n, 256]
  pos[g]     = predicts[g] . hist_y[g]
  neg[g,j]   = predicts[g] . emb[neg_idx[g,j]]   (64 negatives/group)
  loss       = mean_g(logsumexp([pos, neg_g]) - pos)

Sharding: data-parallel over groups, 512 groups/core on 8 cores.  The
embedding table is replicated (negatives index the full table); the
negative-row gather (256 MB total) is done with dma_gather in bf16 (halves
traffic).  Per-core partial loss sums are combined on host.

Gather slot permutation: we are free to choose which (group, j) pair lands
in which gather slot.  Slots are laid out so a chunk of 4096 slots maps to
dst[p, blk, :] with group = (chunk//2)*128 + p and j = (chunk*32)%64 + blk.
Then the predictor row needed by partition p is just row p of the
128-group band -> the multiply's second operand is a plain broadcast AP of
a [128, 256] tile, and per-group negative logits land contiguously in one
partition of the logit tile [128 part, 4 band * 64 j].

Note on the gather: this deployment has no working device-side indexed DMA
(the custom InstDMAGatherAnt Q7 ucode is excluded from the image, and the
stock walrus dynamic-DMA path emits only 16 runtime descriptors — verified
on HW).  The negative-row lookup is therefore resolved on the host while
sharding: the bf16 negative rows are staged per-core in the exact chunk
layout the device consumes, and the kernel streams them sequentially at
full DMA rate (the same bytes a device gather would move).
"""

import os
from contextlib import ExitStack

import numpy as np
import ml_dtypes

N = 4096          # groups
K = 4             # rows per group
H = 256           # embedding dim
M = 64            # negatives per group
NCORES = 8
S = N // NCORES   # 512 groups per core
ROWS = S * K      # 2048 local rows
BANDS = S // 128  # 4 bands of 128 groups
NCHUNK = 8        # gather chunks per core
CH_BLK = (S * M) // (NCHUNK * 128)   # 32 blocks (of 128 slots) per chunk
CH_IDX = CH_BLK * 128                # 4096 gathered rows per chunk

_CACHE = {}


# --------------------------------------------------------------------------
# device program
# --------------------------------------------------------------------------

def build_nc(debug=False):
    import concourse.bass as bass
    import concourse.tile as tile
    from concourse import bacc, masks, mybir

    f32 = mybir.dt.float32
    bf16 = mybir.dt.bfloat16
    i16 = mybir.dt.int16
    Alu = mybir.AluOpType
    Act = mybir.ActivationFunctionType
    Ax = mybir.AxisListType

    nc = bacc.Bacc(
        "TRN2", target_bir_lowering=False, debug=debug, num_devices=NCORES
    )

    embT = nc.dram_tensor("embT", [H, ROWS], f32, kind="ExternalInput").ap()
    histy = nc.dram_tensor("histy", [S, H], f32, kind="ExternalInput").ap()
    Wt = nc.dram_tensor("Wt", [(K - 1) * H, H], f32, kind="ExternalInput").ap()
    bvec = nc.dram_tensor("bvec", [H, 1], f32, kind="ExternalInput").ap()
    negs = nc.dram_tensor(
        "negs", [NCHUNK, 128, CH_BLK, H], bf16, kind="ExternalInput"
    ).ap()
    lossp = nc.dram_tensor("loss_part", [128, 1], f32, kind="ExternalOutput").ap()

    with tile.TileContext(nc) as tc, ExitStack() as ctx:
        cpool = ctx.enter_context(tc.tile_pool(name="const", bufs=1))
        gpool = ctx.enter_context(tc.tile_pool(name="gather", bufs=3))
        ppool = ctx.enter_context(tc.tile_pool(name="prod", bufs=3))
        ipool = ctx.enter_context(tc.tile_pool(name="idx", bufs=2))
        pspool = ctx.enter_context(tc.tile_pool(name="psum", bufs=2, space="PSUM"))
        tpool = ctx.enter_context(tc.tile_pool(name="tps", bufs=2, space="PSUM"))

        # ---- constant loads -------------------------------------------------
        W_sb = []
        for kc in range(6):
            t = cpool.tile([128, H], f32, tag=f"W{kc}")
            nc.sync.dma_start(out=t[:], in_=Wt[128 * kc : 128 * (kc + 1), :])
            W_sb.append(t)
        embT_sb = []
        for hc in range(2):
            t = cpool.tile([128, ROWS], f32, tag=f"embT{hc}")
            nc.sync.dma_start(out=t[:], in_=embT[128 * hc : 128 * (hc + 1), :])
            embT_sb.append(t)
        histy_sb = []
        for B in range(BANDS):
            t = cpool.tile([128, H], f32, tag=f"histy{B}")
            nc.sync.dma_start(out=t[:], in_=histy[128 * B : 128 * (B + 1), :])
            histy_sb.append(t)
        bias_sb = []
        for hc in range(2):
            t = cpool.tile([128, 1], f32, tag=f"bias{hc}")
            nc.sync.dma_start(out=t[:], in_=bvec[128 * hc : 128 * (hc + 1), :])
            bias_sb.append(t)
        ident = cpool.tile([128, 128], f32, tag="ident")
        masks.make_identity(nc, ident[:])

        # ---- predsT = (hist_x @ W + b)^T : [h, g] ---------------------------
        # hist_x^T[j*256+h, g] = embT[h, 4g+j] -> rhs slice of embT_sb.
        predsT_sb = []
        for mc in range(2):
            pt = pspool.tile([128, S], f32, tag="predsT_ps")
            for j in range(K - 1):
                for hc in range(2):
                    kc = 2 * j + hc
                    rhs = embT_sb[hc][:].rearrange("p (g j) -> p j g", j=K)[:, j, :]
                    nc.tensor.matmul(
                        pt[:],
                        lhsT=W_sb[kc][:, 128 * mc : 128 * (mc + 1)],
                        rhs=rhs,
                        start=(kc == 0),
                        stop=(kc == 5),
                    )
            t = cpool.tile([128, S], f32, tag=f"predsT{mc}")
            nc.vector.tensor_scalar_add(t[:], pt[:], bias_sb[mc][:])
            predsT_sb.append(t)

        # ---- transpose preds to [g, h]; bf16 cast; positive logits ----------
        pred16_sb = []
        pos_t = cpool.tile([128, BANDS], f32, tag="pos_t")
        for B in range(BANDS):
            p16 = cpool.tile([128, H], bf16, tag=f"pred16_{B}")
            pprod = cpool.tile([128, H], f32, tag=f"pprod{B}")
            for mc in range(2):
                ps = tpool.tile([128, 128], f32, tag="tps")
                nc.tensor.transpose(
                    ps[:], predsT_sb[mc][:, 128 * B : 128 * (B + 1)], ident[:]
                )
                nc.vector.tensor_copy(p16[:, 128 * mc : 128 * (mc + 1)], ps[:])
                nc.vector.tensor_mul(
                    pprod[:, 128 * mc : 128 * (mc + 1)],
                    ps[:],
                    histy_sb[B][:, 128 * mc : 128 * (mc + 1)],
                )
            nc.vector.tensor_reduce(
                pos_t[:, B : B + 1], pprod[:], axis=Ax.X, op=Alu.add
            )
            pred16_sb.append(p16)

        # ---- negative logits ------------------------------------------------
        nlt = cpool.tile([128, BANDS * M], f32, tag="nlt")
        for ci in range(NCHUNK):
            B = ci // 2
            G = gpool.tile([128, CH_BLK, H], bf16)
            nc.sync.dma_start(out=G[:], in_=negs[ci])
            P = ppool.tile([128, CH_BLK, H], bf16)
            bc = pred16_sb[B][:].unsqueeze(1).broadcast_to([128, CH_BLK, H])
            nc.vector.tensor_tensor(P[:], G[:], bc, op=Alu.mult)
            # h-reduction as a fold tree: tensor_tensor ADD runs in the bf16
            # 2x DVE mode, while InstTensorReduce has no accel uops (1x) —
            # folding halves the reduce cycles.  Intermediate bf16 rounding
            # adds ~0.04 abs noise per logit, ~1e-4 on the final mean loss.
            w = H // 2
            # first (largest) fold on the otherwise-idle GPSIMD engine;
            # remaining folds on DVE (bf16 2x mode)
            nc.gpsimd.tensor_tensor(
                P[:, :, :w], P[:, :, :w], P[:, :, w : 2 * w], op=Alu.add
            )
            while w > 2:
                w //= 2
                nc.vector.tensor_tensor(
                    P[:, :, :w], P[:, :, :w], P[:, :, w : 2 * w], op=Alu.add
                )
            nc.vector.tensor_tensor(
                nlt[:, CH_BLK * ci : CH_BLK * (ci + 1)].unsqueeze(2),
                P[:, :, 0:1],
                P[:, :, 1:2],
                op=Alu.add,
            )

        # ---- per-group logsumexp and loss ----------------------------------
        fpool = ctx.enter_context(tc.tile_pool(name="fin", bufs=1))
        mx = fpool.tile([128, BANDS], f32, tag="mx")
        nc.vector.tensor_reduce(
            mx[:], nlt[:].rearrange("p (b j) -> p b j", b=BANDS),
            axis=Ax.X, op=Alu.max,
        )
        nc.vector.tensor_tensor(mx[:], mx[:], pos_t[:], op=Alu.max)
        negmx = fpool.tile([128, BANDS], f32, tag="negmx")
        nc.vector.tensor_scalar_mul(negmx[:], mx[:], -1.0)
        sume = fpool.tile([128, BANDS], f32, tag="sume")
        scr = fpool.tile([128, M], f32, tag="scr")
        for B in range(BANDS):
            nc.scalar.activation(
                scr[:],
                nlt[:, M * B : M * (B + 1)],
                Act.Exp,
                bias=negmx[:, B : B + 1],
                accum_out=sume[:, B : B + 1],
            )
        pd = fpool.tile([128, BANDS], f32, tag="pd")
        nc.vector.tensor_tensor(pd[:], pos_t[:], mx[:], op=Alu.subtract)
        pexp = fpool.tile([128, BANDS], f32, tag="pexp")
        nc.scalar.activation(pexp[:], pd[:], Act.Exp)
        tot = fpool.tile([128, BANDS], f32, tag="tot")
        nc.vector.tensor_tensor(tot[:], sume[:], pexp[:], op=Alu.add)
        lse = fpool.tile([128, BANDS], f32, tag="lse")
        nc.scalar.activation(lse[:], tot[:], Act.Ln)
        # loss_pg = lse + mx - pos
        nc.vector.tensor_tensor(lse[:], lse[:], mx[:], op=Alu.add)
        nc.vector.tensor_tensor(lse[:], lse[:], pos_t[:], op=Alu.subtract)
        lred = fpool.tile([128, 1], f32, tag="lred")
        nc.vector.tensor_reduce(lred[:], lse[:], axis=Ax.X, op=Alu.add)
        nc.sync.dma_start(out=lossp, in_=lred[:])

    nc.compile()
    return nc


# --------------------------------------------------------------------------
# host-side sharding
# --------------------------------------------------------------------------

def _neg_indices(target, perm, k, m):
    """neg_idx[g, j] = cand[g][perm[g, j]] exactly as the reference builds it."""
    n = target.shape[0] // k
    t64 = np.asarray(target)
    expected = np.repeat(np.arange(n, dtype=t64.dtype), k)
    p = np.asarray(perm)[:, :m].astype(np.int64)
    if np.array_equal(t64, expected):
        # cand[g][j] = j if j < k*g else j + k
        g = np.arange(n, dtype=np.int64)[:, None]
        return p + k * (p >= k * g)
    # generic (slow) fallback, matches jnp.where(..., size=k*(n-1), fill=0)
    group_t = t64[0::k]
    out = np.zeros((n, m), dtype=np.int64)
    order = np.arange(t64.shape[0], dtype=np.int64)
    for gi in range(n):
        cand = order[t64 != group_t[gi]]
        cand = np.pad(cand, (0, k * (n - 1) - cand.shape[0]))
        out[gi] = cand[p[gi]]
    return out


def _prep_inputs(embeddings, W, b, target, perm, k, m):
    emb = np.ascontiguousarray(np.asarray(embeddings, dtype=np.float32))
    emb16 = emb.astype(ml_dtypes.bfloat16)
    Wf = np.ascontiguousarray(np.asarray(W, dtype=np.float32))
    bf = np.asarray(b, dtype=np.float32).reshape(H, 1)
    neg_idx = _neg_indices(target, perm, k, m)  # [N, M]

    in_maps = []
    for c in range(NCORES):
        sl = emb[ROWS * c : ROWS * (c + 1)]
        embT = np.ascontiguousarray(sl.T)
        hy = np.ascontiguousarray(sl[K - 1 :: K])
        # negative rows staged in the chunk layout the device consumes:
        # negs[ci, p, blk, :] = emb16[neg_idx[g, j]] with
        # g = (ci//2)*128 + p (local), j = (ci*CH_BLK) % M + blk.
        ni = neg_idx[S * c : S * (c + 1)]  # [S, M]
        blk = np.arange(CH_BLK)
        p = np.arange(128)
        rows = np.empty((NCHUNK, 128, CH_BLK), dtype=np.int64)
        for ci in range(NCHUNK):
            B = ci // 2
            g_local = B * 128 + p[:, None]
            j = (ci * CH_BLK) % M + blk[None, :]
            rows[ci] = ni[g_local, j]
        ng = emb16[rows.reshape(-1)].reshape(NCHUNK, 128, CH_BLK, H)
        in_maps.append(
            {
                "embT": embT,
                "histy": hy,
                "Wt": Wf,
                "bvec": bf,
                "negs": ng,
            }
        )
    return in_maps


def kernel(embeddings, W, b, target, perm, k_pos_samples, m_neg_samples):
    k = int(k_pos_samples)
    m = min(int(m_neg_samples), k * (N - 1))
    assert k == K and m == M and embeddings.shape == (N * K, H)

    if "nc" not in _CACHE:
        _CACHE["nc"] = build_nc(debug=False)
    nc = _CACHE["nc"]

    in_maps = _prep_inputs(embeddings, W, b, target, perm, k, m)

    from concourse.bass_utils import run_bass_kernel_spmd

    res = run_bass_kernel_spmd(nc, in_maps, list(range(NCORES)))
    total = 0.0
    for c in range(NCORES):
        total += float(np.sum(res.results[c]["loss_part"].astype(np.float64)))
    return np.float32(total / N)



# revision 2
# speedup vs baseline: 1.0910x; 1.0910x over previous
"""CPC loss (nn_CPCLossV2) Trainium2 Bass kernel — minimal-wire version.

Problem: n=4096 groups x k=4 rows of h=256 embeddings.
  hist_x[g]  = rows 4g..4g+2 concat -> [n, 768]
  predicts   = hist_x @ W + b       -> [n, 256]
  logits[g]  = [predicts[g].emb[4g+3], predicts[g].emb[neg_idx[g, 0..63]]]
  loss       = mean_g(logsumexp(logits_g) - logits_g[0])

Host->device transfer over the axon tunnel dominates the wall clock, so
the kernel is architected to minimize wire traffic (~0.62MB/core instead
of the 20MB/core a host-side-gather design needs):

  * embeddings are shipped SHARDED in fp8 e4m3 (2048 rows/core,
    transposed) and AllGathered on-device over the device fabric.
  * W is shipped bf16, sharded by output column (32/core), AllGathered.
  * ALL 16384 logits per group are computed on-device by PE matmul
    (l^T[r, g] = sum_h embT[h, r] * predsT[h, g]) into a bf16 tile
    LT[hi, g, lo] with r = 128*lo + hi.
  * the 65 needed logits per group (positive at j=0, 64 negatives) are
    selected on-device with a two-level one-hot gather (no GPSIMD custom
    ucode on this image): a per-group one-hot matmul picks partition hi,
    then an is_equal mask + reduce picks lo.  Host ships only the hi/lo
    index planes as uint8 (66KB/core).
  * fp8 quantization of the table perturbs the loss by ~7e-4 relative
    (validated against the fp32 reference in sim and on HW); the gate is
    2e-2.
  * per-group logsumexp epilogue identical to the reference; per-core
    partial sums returned as [128, 1] and combined on host.
"""

import os
from contextlib import ExitStack

import numpy as np
import ml_dtypes

N = 4096          # groups
K = 4             # rows per group
H = 256           # embedding dim
M = 64            # negatives per group
J = M + 1         # selections per group (j=0 is the positive)
NCORES = 8
S = N // NCORES   # 512 groups per core
ROWS = S * K      # 2048 local rows
RT = N * K        # 16384 total rows
BANDS = S // 128  # 4 bands of 128 groups
GB = 7            # hiRep psum batch: GB*J = 455 <= 512 fp32 psum cols
WSH = H // NCORES  # 32 W columns per core

AGE_EMB = H * ROWS            # 524288 bf16 elems
AGE_W = (K - 1) * H * WSH     # 24576
AGE = AGE_EMB + AGE_W         # 548864 = 268 * 2048
AGROWS = AGE // ROWS          # 268

_CACHE = {}


# --------------------------------------------------------------------------
# device program
# --------------------------------------------------------------------------

def build_nc(debug=False):
    import concourse.bass as bass
    import concourse.tile as tile
    from concourse import bacc, masks, mybir

    f32 = mybir.dt.float32
    bf16 = mybir.dt.bfloat16
    Alu = mybir.AluOpType
    Act = mybir.ActivationFunctionType
    Ax = mybir.AxisListType

    nc = bacc.Bacc(
        "TRN2", target_bir_lowering=False, debug=debug, num_devices=NCORES
    )

    f8 = mybir.dt.float8e4
    u8 = mybir.dt.uint8
    embT_sh = nc.dram_tensor("embT_sh", [H, ROWS], f8, kind="ExternalInput").ap()
    Wsh = nc.dram_tensor("Wsh", [(K - 1) * H, WSH], bf16, kind="ExternalInput").ap()
    bvec = nc.dram_tensor("bvec", [H, 1], f32, kind="ExternalInput").ap()
    idxhi = nc.dram_tensor("idxhi", [1, S * J], u8, kind="ExternalInput").ap()
    idxloT = nc.dram_tensor("idxloT", [J, S], u8, kind="ExternalInput").ap()
    ones_in = nc.dram_tensor("ones_in", [1, 128], bf16, kind="ExternalInput").ap()
    iotaf4 = nc.dram_tensor("iotaf4", [1, 512], bf16, kind="ExternalInput").ap()
    lossp = nc.dram_tensor("loss_part", [128, 1], f32, kind="ExternalOutput").ap()

    with tile.TileContext(nc) as tc, ExitStack() as ctx:
        dram = ctx.enter_context(tc.tile_pool(name="dram", bufs=1, space="DRAM"))
        cpool = ctx.enter_context(tc.tile_pool(name="const", bufs=1))
        oh1pool = ctx.enter_context(tc.tile_pool(name="oh1", bufs=2))
        zpool = ctx.enter_context(tc.tile_pool(name="zp", bufs=2))
        pslog = ctx.enter_context(tc.tile_pool(name="pslog", bufs=2, space="PSUM"))
        psh = ctx.enter_context(tc.tile_pool(name="psh", bufs=2, space="PSUM"))
        psT = ctx.enter_context(tc.tile_pool(name="psT", bufs=2, space="PSUM"))
        psz = ctx.enter_context(tc.tile_pool(name="psz", bufs=2, space="PSUM"))

        # ---- all-gather emb + W shards over the device fabric ---------------
        agin_e = dram.tile([H, ROWS], f8, tag="agin_e")
        agout_e = dram.tile([NCORES, H, ROWS], f8, tag="agout_e")
        agin_w = dram.tile([AGE_W // ROWS, ROWS], bf16, tag="agin_w")
        agout_w = dram.tile([NCORES, AGE_W // ROWS, ROWS], bf16, tag="agout_w")
        nc.sync.dma_start(out=agin_e[:], in_=embT_sh)
        nc.sync.dma_start(
            out=agin_w[:],
            in_=Wsh.rearrange("(a b) w -> a (b w)", a=AGE_W // ROWS),
        )
        nc.gpsimd.collective_compute(
            "AllGather",
            Alu.bypass,
            replica_groups=[list(range(NCORES))],
            ins=[agin_e.opt()],
            outs=[agout_e.opt()],
        )
        nc.gpsimd.collective_compute(
            "AllGather",
            Alu.bypass,
            replica_groups=[list(range(NCORES))],
            ins=[agin_w.opt()],
            outs=[agout_w.opt()],
        )

        # ---- SBUF constant loads -------------------------------------------
        embT_loc = []
        for hc in range(2):
            t = cpool.tile([128, ROWS], f8, tag=f"embT_loc{hc}")
            nc.sync.dma_start(out=t[:], in_=embT_sh[128 * hc : 128 * (hc + 1), :])
            embT_loc.append(t)
        bias_sb = []
        for mc in range(2):
            t = cpool.tile([128, 1], f32, tag=f"bias{mc}")
            nc.sync.dma_start(out=t[:], in_=bvec[128 * mc : 128 * (mc + 1), :])
            bias_sb.append(t)
        ihpool = ctx.enter_context(tc.tile_pool(name="ihp", bufs=2))
        idxloT_u8 = cpool.tile([J, S], u8, tag="idxloT_u8")
        nc.sync.dma_start(out=idxloT_u8[:], in_=idxloT)
        idxloT_sb = cpool.tile([J, S], bf16, tag="idxloT_sb")
        nc.vector.tensor_copy(idxloT_sb[:], idxloT_u8[:])
        ones_sb = cpool.tile([1, 128], bf16, tag="ones_sb")
        nc.sync.dma_start(out=ones_sb[:], in_=ones_in)
        iotaf4_sb = cpool.tile([1, 512], bf16, tag="iotaf4_sb")
        nc.sync.dma_start(out=iotaf4_sb[:], in_=iotaf4)
        iotap_sb = cpool.tile([128, 512], bf16, tag="iotap_sb")
        nc.gpsimd.iota(
            iotap_sb[:], [[0, 512]], channel_multiplier=1,
            allow_small_or_imprecise_dtypes=True,
        )
        ident = cpool.tile([128, 128], f32, tag="ident")
        masks.make_identity(nc, ident[:])

        # gathered full table / W: wait on agout then spread into SBUF
        embT_full = []
        ag_emb = agout_e[:].rearrange("c h r -> h c r")
        for hc in range(2):
            t = cpool.tile([128, RT], f8, tag=f"embT_full{hc}")
            nc.sync.dma_start(
                out=t[:].rearrange("p (c r) -> p c r", c=NCORES),
                in_=ag_emb[128 * hc : 128 * (hc + 1)],
            )
            embT_full.append(t)
        ag_w = agout_w[:].rearrange(
            "c a (b w) -> (a b) c w", w=WSH
        )
        W_full = []
        for kc in range(6):
            t = cpool.tile([128, H], bf16, tag=f"W_full{kc}")
            nc.sync.dma_start(
                out=t[:].rearrange("p (c w) -> p c w", c=NCORES),
                in_=ag_w[128 * kc : 128 * (kc + 1)],
            )
            W_full.append(t)

        # ---- predsT = (hist_x @ W + b)^T : [h, g] in bf16 -------------------
        predsT = []
        for mc in range(2):
            pt = psz.tile([128, S], f32, tag="ps_misc")
            for j in range(K - 1):
                for hc in range(2):
                    kc = 2 * j + hc
                    rhs = embT_loc[hc][:].rearrange("p (g j) -> p j g", j=K)[:, j, :]
                    nc.tensor.matmul(
                        pt[:],
                        lhsT=W_full[kc][:, 128 * mc : 128 * (mc + 1)],
                        rhs=rhs,
                        start=(kc == 0),
                        stop=(kc == 5),
                    )
            t = cpool.tile([128, S], bf16, tag=f"predsT{mc}")
            nc.vector.tensor_scalar_add(t[:], pt[:], bias_sb[mc][:])
            predsT.append(t)

        # ---- iota along lo, replicated on 65 partitions / 4 group slots -----
        ps_i = psz.tile([J, 512], f32, tag="ps_misc")
        nc.tensor.matmul(
            ps_i[:], lhsT=ones_sb[:, 0:J], rhs=iotaf4_sb[:], start=True, stop=True
        )
        iota65_4 = cpool.tile([J, 4, 128], f32, tag="iota65_4")
        nc.vector.tensor_copy(iota65_4[:], ps_i[:].rearrange("p (a b) -> p a b", a=4))

        # ---- per-band: logits, two-level gather, logsumexp ------------------
        LT = cpool.tile([128, 128, 128], bf16, tag="LT")  # [hi, g, lo]
        mx_b = cpool.tile([128, BANDS], f32, tag="mx_b")
        sume_b = cpool.tile([128, BANDS], f32, tag="sume_b")
        pos_b = cpool.tile([128, BANDS], f32, tag="pos_b")
        scr = cpool.tile([128, J], f32, tag="scr")

        for B in range(BANDS):
            # all 16384 logits for the band's 128 groups
            for lo in range(128):
                pl = pslog.tile([128, 128], f32, tag="ps_log")
                for mc in range(2):
                    nc.tensor.matmul(
                        pl[:],
                        lhsT=embT_full[mc][:, 128 * lo : 128 * (lo + 1)],
                        rhs=predsT[mc][:, 128 * B : 128 * (B + 1)],
                        start=(mc == 0),
                        stop=(mc == 1),
                    )
                nc.vector.tensor_copy(LT[:, :, lo], pl[:])

            # one-hot over hi for every (group, j) of the band
            ihb_u8 = ihpool.tile([1, 128 * J], u8, tag="ihb_u8")
            nc.sync.dma_start(
                out=ihb_u8[:], in_=idxhi[:, B * 128 * J : (B + 1) * 128 * J]
            )
            ihb = ihpool.tile([1, 128 * J], bf16, tag="ihb")
            nc.vector.tensor_copy(ihb[:], ihb_u8[:])
            OH1 = oh1pool.tile([128, 128 * J], bf16, tag="OH1")
            for gb in range(0, 128, GB):
                nb = min(GB, 128 - gb)
                ph = psh.tile([128, GB * J], f32, tag="ps_hi")
                nc.tensor.matmul(
                    ph[:, 0 : nb * J],
                    lhsT=ones_sb[:],
                    rhs=ihb[:, gb * J : (gb + nb) * J],
                    start=True,
                    stop=True,
                )
                nc.vector.tensor_tensor(
                    OH1[:, gb * J : (gb + nb) * J],
                    ph[:, 0 : nb * J],
                    iotap_sb[:, 0 : nb * J],
                    op=Alu.is_equal,
                )

            # T[j, lo] = LT[hi_j, g, lo] via one-hot matmul; pick lo by mask
            Z = zpool.tile([J, 128], f32, tag="Z")
            for q4 in range(0, 128, 4):
                pT = psT.tile([J, 4, 128], f32, tag="ps_T")
                for u in range(4):
                    g = q4 + u
                    nc.tensor.matmul(
                        pT[:, u, :],
                        lhsT=OH1[:, g * J : (g + 1) * J],
                        rhs=LT[:, g, :],
                        start=True,
                        stop=True,
                    )
                oh2 = zpool.tile([J, 4, 128], bf16, tag="oh2")
                locols = idxloT_sb[:, B * 128 + q4 : B * 128 + q4 + 4]
                nc.vector.tensor_tensor(
                    oh2[:],
                    iota65_4[:],
                    locols.unsqueeze(2).broadcast_to([J, 4, 128]),
                    op=Alu.is_equal,
                )
                prod = zpool.tile([J, 4, 128], f32, tag="prod")
                nc.vector.tensor_tensor(prod[:], pT[:], oh2[:], op=Alu.mult)
                nc.vector.tensor_reduce(
                    Z[:, q4 : q4 + 4], prod[:], axis=Ax.X, op=Alu.add
                )

            # transpose Z -> [g, j]; logsumexp pieces
            pz = psz.tile([128, J], f32, tag="ps_misc")
            nc.tensor.transpose(pz[:], Z[:], ident[0:J, 0:J])
            nc.vector.tensor_reduce(
                mx_b[:, B : B + 1], pz[:], axis=Ax.X, op=Alu.max
            )
            negmx = zpool.tile([128, 1], f32, tag="negmx")
            nc.vector.tensor_scalar_mul(negmx[:], mx_b[:, B : B + 1], -1.0)
            nc.scalar.activation(
                scr[:],
                pz[:],
                Act.Exp,
                bias=negmx[:],
                accum_out=sume_b[:, B : B + 1],
            )
            nc.vector.tensor_copy(pos_b[:, B : B + 1], pz[:, 0:1])

        # ---- loss_pg = ln(sume) + mx - pos; partial sum out -----------------
        lse = cpool.tile([128, BANDS], f32, tag="lse")
        nc.scalar.activation(lse[:], sume_b[:], Act.Ln)
        nc.vector.tensor_tensor(lse[:], lse[:], mx_b[:], op=Alu.add)
        nc.vector.tensor_tensor(lse[:], lse[:], pos_b[:], op=Alu.subtract)
        lred = cpool.tile([128, 1], f32, tag="lred")
        nc.vector.tensor_reduce(lred[:], lse[:], axis=Ax.X, op=Alu.add)
        nc.sync.dma_start(out=lossp, in_=lred[:])

    nc.compile()
    return nc


# --------------------------------------------------------------------------
# host-side sharding
# --------------------------------------------------------------------------

def _neg_indices(target, perm, k, m):
    """neg_idx[g, j] = cand[g][perm[g, j]] exactly as the reference builds it."""
    n = target.shape[0] // k
    t64 = np.asarray(target)
    expected = np.repeat(np.arange(n, dtype=t64.dtype), k)
    p = np.asarray(perm)[:, :m].astype(np.int64)
    if np.array_equal(t64, expected):
        # cand[g][j] = j if j < k*g else j + k
        g = np.arange(n, dtype=np.int64)[:, None]
        return p + k * (p >= k * g)
    # generic (slow) fallback, matches jnp.where(..., size=k*(n-1), fill=0)
    group_t = t64[0::k]
    out = np.zeros((n, m), dtype=np.int64)
    order = np.arange(t64.shape[0], dtype=np.int64)
    for gi in range(n):
        cand = order[t64 != group_t[gi]]
        cand = np.pad(cand, (0, k * (n - 1) - cand.shape[0]))
        out[gi] = cand[p[gi]]
    return out


def _prep_inputs(embeddings, W, b, target, perm, k, m):
    bf16 = ml_dtypes.bfloat16
    f8 = ml_dtypes.float8_e4m3
    emb = np.asarray(embeddings, dtype=np.float32)
    Wf = np.asarray(W, dtype=np.float32).astype(bf16)
    bf = np.asarray(b, dtype=np.float32).reshape(H, 1)
    neg_idx = _neg_indices(target, perm, k, m)  # [N, M] global rows

    # selection table: j=0 positive row (4g+3), then the 64 negatives
    gidx = np.arange(N, dtype=np.int64)
    sel = np.empty((N, J), dtype=np.int64)
    sel[:, 0] = K * gidx + (K - 1)
    sel[:, 1:] = neg_idx
    hi = (sel % 128).astype(np.uint8)
    lo = (sel // 128).astype(np.uint8)

    ones = np.ones((1, 128), dtype=bf16)
    iotaf4 = np.tile(np.arange(128, dtype=np.float32), 4)[None, :].astype(bf16)

    in_maps = []
    for c in range(NCORES):
        sl = emb[ROWS * c : ROWS * (c + 1)]
        embT = np.ascontiguousarray(sl.T.astype(f8))
        hic = np.ascontiguousarray(hi[S * c : S * (c + 1)].reshape(1, S * J))
        loc = np.ascontiguousarray(lo[S * c : S * (c + 1)].T)
        in_maps.append(
            {
                "embT_sh": embT,
                "Wsh": np.ascontiguousarray(Wf[:, WSH * c : WSH * (c + 1)]),
                "bvec": bf,
                "idxhi": hic,
                "idxloT": loc,
                "ones_in": ones,
                "iotaf4": iotaf4,
            }
        )
    return in_maps


def _run(nc, in_maps):
    """Execute the compiled module on the 8 cores.

    Same lowering as bass_utils.run_bass_kernel_spmd's axon path
    (bass2jax.run_bass_via_pjrt), but the jit-wrapped shard_map callable is
    built ONCE and cached: run_bass_kernel_spmd re-creates the closure per
    call, which forces a full jax retrace + XLA rebuild (~0.3s) on every
    invocation even though the NEFF itself is cached.  Falls back to
    run_bass_kernel_spmd if the fast path can't initialize.
    """
    if "runner" not in _CACHE:
        try:
            _CACHE["runner"] = _make_runner(nc)
        except Exception:
            _CACHE["runner"] = None
    runner = _CACHE["runner"]
    if runner is not None:
        return runner(in_maps)

    from concourse.bass_utils import run_bass_kernel_spmd

    res = run_bass_kernel_spmd(nc, in_maps, list(range(NCORES)))
    return [res.results[c] for c in range(NCORES)]


def _make_runner(nc):
    import jax
    from jax.sharding import Mesh, PartitionSpec
    try:
        from jax.experimental.shard_map import shard_map
    except ImportError:
        from jax import shard_map
    from concourse import mybir
    from concourse.bass2jax import (
        _bass_exec_p,
        install_neuronx_cc_hook,
        partition_id_tensor,
    )
    from concourse.bass_utils import axon_active

    if not axon_active():
        return None
    install_neuronx_cc_hook()
    assert nc.dbg_addr is None

    partition_name = (
        nc.partition_id_tensor.name if nc.partition_id_tensor else None
    )
    in_names, out_names, out_avals = [], [], []
    for alloc in nc.m.functions[0].allocations:
        if not isinstance(alloc, mybir.MemoryLocationSet):
            continue
        name = alloc.memorylocations[0].name
        if alloc.kind == "ExternalInput":
            if name != partition_name:
                in_names.append(name)
        elif alloc.kind == "ExternalOutput":
            out_names.append(name)
            out_avals.append(
                jax.core.ShapedArray(
                    tuple(alloc.tensor_shape), mybir.dt.np(alloc.dtype)
                )
            )
    n_params = len(in_names)
    n_outs = len(out_names)
    all_names = list(in_names) + list(out_names)
    if partition_name is not None:
        all_names.append(partition_name)

    def _body(*args):
        operands = list(args)
        if partition_name is not None:
            operands.append(partition_id_tensor())
        return tuple(
            _bass_exec_p.bind(
                *operands,
                out_avals=tuple(out_avals),
                in_names=tuple(all_names),
                out_names=tuple(out_names),
                lowering_input_output_aliases=(),
                sim_require_finite=True,
                sim_require_nnan=True,
                nc=nc,
            )
        )

    devices = jax.devices()[:NCORES]
    mesh = Mesh(np.asarray(devices), ("core",))
    donate = tuple(range(n_params, n_params + n_outs))
    sharded = jax.jit(
        shard_map(
            _body,
            mesh=mesh,
            in_specs=(PartitionSpec("core"),) * (n_params + n_outs),
            out_specs=(PartitionSpec("core"),) * n_outs,
            check_rep=False,
        ),
        donate_argnums=donate,
        keep_unused=True,
    )

    def runner(in_maps):
        concat_in = [
            np.concatenate([np.asarray(m[name]) for m in in_maps], axis=0)
            for name in in_names
        ]
        concat_zeros = [
            np.zeros((NCORES * a.shape[0], *a.shape[1:]), a.dtype)
            for a in out_avals
        ]
        out_arrs = sharded(*concat_in, *concat_zeros)
        return [
            {
                name: np.asarray(out_arrs[i]).reshape(
                    NCORES, *out_avals[i].shape
                )[c]
                for i, name in enumerate(out_names)
            }
            for c in range(NCORES)
        ]

    return runner


def kernel(embeddings, W, b, target, perm, k_pos_samples, m_neg_samples):
    k = int(k_pos_samples)
    m = min(int(m_neg_samples), k * (N - 1))
    assert k == K and m == M and embeddings.shape == (N * K, H)

    if "nc" not in _CACHE:
        _CACHE["nc"] = build_nc(debug=False)
    nc = _CACHE["nc"]

    in_maps = _prep_inputs(embeddings, W, b, target, perm, k, m)

    results = _run(nc, in_maps)
    total = 0.0
    for c in range(NCORES):
        total += float(np.sum(results[c]["loss_part"].astype(np.float64)))
    return np.float32(total / N)
